# revision 5
# baseline (speedup 1.0000x reference)
"""Trainium2 kernel for nn_LocScaleTransform (bilinear spatial sampler).

Math: out[b,i,j,c] = sum_{h,w} Y_b[i,h] * X_b[j,w] * img[b,h,w,c]
where Y/X are per-example bilinear interpolation matrices (2 nonzeros/row),
computed on the host from `transforms` (48 floats of input).

Device pipeline per example (all heavy math on the PE array, float32r):
  stage Y : YI[i,(w,c)]   = Yt.T @ img          (k=h, rhs natural layout)
  transp  : YIT[w,(c,i)]  = PE-transpose of YI  (per-c 128x128 tiles)
  stage X : out[i,(j,c)]  = (YIT_c).T @ Xt ...  (k=w, m=i, n=j per channel)

Sharding: pure data parallel, 2 examples per core x 8 cores.
"""
import sys

sys.path.insert(0, "/opt/trn_rl_repo")

import numpy as np
from contextlib import ExitStack

B, H, W, C = 16, 256, 256, 64
FREE = W * C          # 16384 = per-row free elements (w,c)
NJ = H                # output cols (min_hw)
NCORES, EX = 8, 2
P = 128

_prog_cache = {}


def _host_tables(transforms):
    """Per-example interp matrices Yt [B,H,256] (h,i) and Xt [B,W,256] (w,j), fp32."""
    t = np.asarray(transforms, dtype=np.float32)
    lin = np.linspace(np.float32(-1.0), np.float32(1.0), H).astype(np.float32)
    one = np.float32(1.0)
    half = np.float32(H * 0.5)
    scale = t[:, 2] + one            # (B,)
    tx = t[:, 0]
    ty = t[:, 1]

    def interp_matrix(trans):
        # coords for output index k: (lin[k]*scale + trans + 1) * 128  (fp32 ops)
        v = (lin[None, :] * scale[:, None] + trans[:, None] + one) * half  # (B,256)
        v0f = np.floor(v)
        v1f = v0f + one
        v0f = np.clip(v0f, 0.0, np.float32(H - 1))
        v1f = np.clip(v1f, 0.0, np.float32(H - 1))
        w0 = (v1f - v).astype(np.float32)
        w1 = (v - v0f).astype(np.float32)
        same = v0f == v1f            # fully clipped -> exact zero in reference
        w0 = np.where(same, np.float32(0.0), w0)
        w1 = np.where(same, np.float32(0.0), w1)
        i0 = v0f.astype(np.int64)
        i1 = v1f.astype(np.int64)
        m = np.zeros((t.shape[0], H, H), dtype=np.float32)  # [b, out k, src]
        bidx = np.arange(t.shape[0])[:, None]
        kidx = np.arange(H)[None, :]
        np.add.at(m, (bidx, kidx, i0), w0)
        np.add.at(m, (bidx, kidx, i1), w1)
        return m

    Y = interp_matrix(ty)            # [B, i, h]
    X = interp_matrix(tx)            # [B, j, w]
    Yt = np.ascontiguousarray(np.swapaxes(Y, 1, 2))  # [B, h, i]
    Xt = np.ascontiguousarray(np.swapaxes(X, 1, 2))  # [B, w, j]
    return Yt, Xt


def _build_program(repeat=1):
    import concourse.tile as tile
    from concourse import bacc, mybir

    f32 = mybir.dt.float32
    f32r = mybir.dt.float32r

    nc = bacc.Bacc("TRN2", target_bir_lowering=False, debug=False)
    img_d = nc.dram_tensor("img", [EX, H, FREE], f32r, kind="ExternalInput").ap()
    yt_d = nc.dram_tensor("yt", [EX, H, H], f32r, kind="ExternalInput").ap()
    xt_d = nc.dram_tensor("xt", [EX, W, NJ], f32r, kind="ExternalInput").ap()
    id_d = nc.dram_tensor("ident", [P, P], f32r, kind="ExternalInput").ap()
    out_d = nc.dram_tensor("out", [EX, H, NJ * C], f32, kind="ExternalOutput").ap()

    with tile.TileContext(nc) as tc, ExitStack() as ctx:
        tabs = ctx.enter_context(tc.tile_pool(name="tabs", bufs=1))
        imgp = ctx.enter_context(tc.tile_pool(name="imgp", bufs=2))
        yip = ctx.enter_context(tc.tile_pool(name="yip", bufs=1))
        yitp = ctx.enter_context(tc.tile_pool(name="yitp", bufs=1))
        ypsum = ctx.enter_context(tc.tile_pool(name="ypsum", bufs=4, space="PSUM"))
        tpsum = ctx.enter_context(tc.tile_pool(name="tpsum", bufs=2, space="PSUM"))
        xpsum = ctx.enter_context(tc.tile_pool(name="xpsum", bufs=2, space="PSUM"))

        ident = tabs.tile([P, P], f32r, tag="ident")
        nc.sync.dma_start(ident[:], id_d)

        for rep in range(repeat):
            for e in range(EX):
                yt_sb = tabs.tile([P, 2 * H], f32r, tag="yt")
                xt_sb = tabs.tile([P, 2 * NJ], f32r, tag="xt")
                for kt in range(2):
                    nc.sync.dma_start(yt_sb[:, kt * H:(kt + 1) * H],
                                      yt_d[e, kt * P:(kt + 1) * P, :])
                    nc.sync.dma_start(xt_sb[:, kt * NJ:(kt + 1) * NJ],
                                      xt_d[e, kt * P:(kt + 1) * P, :])

                yit = [yitp.tile([P, C * H], f32r, tag=f"yit{wt}", name=f"yit{wt}") for wt in range(2)]

                # ---- Phase A: stage Y + transpose, per w-slab ----
                for ws in range(2):
                    yi = [yip.tile([P, 8192], f32r, tag=f"yi{it}", name=f"yi{it}") for it in range(2)]
                    for ch in range(16):
                        # free-dim slice of this chunk within the full row
                        f0 = ws * 8192 + ch * 512
                        ims = []
                        for kt in range(2):
                            im = imgp.tile([P, 512], f32r, tag=f"img{kt}")
                            nc.sync.dma_start(
                                im[:], img_d[e, kt * P:(kt + 1) * P, f0:f0 + 512])
                            ims.append(im)
                        for it in range(2):
                            ps = ypsum.tile([P, 512], f32, tag="yps")
                            for kt in range(2):
                                nc.tensor.matmul(
                                    ps[:],
                                    yt_sb[:, kt * H + it * P: kt * H + it * P + P],
                                    ims[kt][:],
                                    start=(kt == 0), stop=(kt == 1))
                            if ch % 2 == 0:
                                nc.vector.tensor_copy(yi[it][:, ch * 512:(ch + 1) * 512], ps[:])
                            else:
                                nc.scalar.copy(yi[it][:, ch * 512:(ch + 1) * 512], ps[:])
                    # transposes: per c, 128x128 [i x w] -> [w x i]
                    for it in range(2):
                        yiv = yi[it][:].rearrange("p (w c) -> p w c", c=C)
                        for cq in range(16):
                            tp = tpsum.tile([P, 512], f32r, tag="tps")
                            for q in range(4):
                                c = cq * 4 + q
                                nc.tensor.transpose(
                                    tp[:, q * P:(q + 1) * P], yiv[:, :, c], ident[:])
                            dst = yit[ws][:].rearrange(
                                "p (c i) -> p c i", i=H)[:, cq * 4:(cq + 1) * 4,
                                                         it * P:(it + 1) * P]
                            src = tp[:].rearrange("p (q i) -> p q i", i=P)
                            if cq % 2 == 0:
                                nc.vector.tensor_copy(dst, src)
                            else:
                                nc.scalar.copy(dst, src)

                # ---- Phase B: stage X ----
                for it in range(2):
                    outh = [yip.tile([P, 8192], f32, tag=f"yi{jh}", name=f"outh{jh}") for jh in range(2)]
                    for cp in range(32):
                        xps = xpsum.tile([P, 512], f32, tag="xps")
                        for c2 in range(2):
                            c = cp * 2 + c2
                            for wt in range(2):
                                nc.tensor.matmul(
                                    xps[:, c2 * NJ:(c2 + 1) * NJ],
                                    yit[wt][:, c * H + it * P: c * H + it * P + P],
                                    xt_sb[:, wt * NJ:(wt + 1) * NJ],
                                    start=(wt == 0), stop=(wt == 1))
                        # evict: src [p, (c2,j)] -> dst out[(j,c)] at c=cp*2
                        for jh in range(2):
                            src = xps[:].rearrange(
                                "p (c j) -> p j c", j=NJ)[:, jh * P:(jh + 1) * P, :]
                            dst = outh[jh][:].rearrange(
                                "p (j c) -> p j c", c=C)[:, :, cp * 2:cp * 2 + 2]
                            if cp % 2 == 0:
                                nc.vector.tensor_copy(dst, src)
                            else:
                                nc.scalar.copy(dst, src)
                    for jh in range(2):
                        nc.sync.dma_start(
                            out_d[e, it * P:(it + 1) * P,
                                  jh * 8192:(jh + 1) * 8192], outh[jh][:])

    nc.compile()
    return nc


def _get_program(repeat=1):
    if repeat not in _prog_cache:
        _prog_cache[repeat] = _build_program(repeat)
    return _prog_cache[repeat]


def run_on_device(transforms, imgs, repeat=1):
    from concourse.bass_utils import run_bass_kernel_spmd

    Yt, Xt = _host_tables(transforms)
    imgs = np.ascontiguousarray(np.asarray(imgs, dtype=np.float32))
    imgs_r = imgs.reshape(B, H, FREE)
    ident = np.eye(P, dtype=np.float32)

    in_maps = []
    for core in range(NCORES):
        b0 = core * EX
        in_maps.append({
            "img": imgs_r[b0:b0 + EX],
            "yt": Yt[b0:b0 + EX],
            "xt": Xt[b0:b0 + EX],
            "ident": ident,
        })
    nc = _get_program(repeat)
    res = run_bass_kernel_spmd(nc, in_maps, core_ids=list(range(NCORES)))
    out = np.empty((B, H, NJ, C), dtype=np.float32)
    for core in range(NCORES):
        b0 = core * EX
        out[b0:b0 + EX] = res.results[core]["out"].reshape(EX, H, NJ, C)
    return out


def kernel(transforms, imgs):
    return run_on_device(transforms, imgs, repeat=1)
